# revision 3
# baseline (speedup 1.0000x reference)
"""Trainium2 Bass kernel for nn_BatchedCauchyKernel (fp8 DoubleRow version).

Computes, for x[N,D], y[M,D], sample_x[N,S], sample_y[M,S], scale[S]:
    d[i,j]   = |x_i|^2 + |y_j|^2 - 2 x_i.y_j
    sx_i     = clip(softplus(sample_x_i . scale), 1e-10, 1e4)
    sy_j     = clip(softplus(sample_y_j . scale), 1e-10, 1e4)
    res      = 1 / (1 + d / sqrt(sx_i * sy_j))
    out      = res * sigmoid(phi * (res - clip(cutoff, 0, 1000)))

Sharding: 2D grid over 8 cores, 4 x-blocks (NS=2048) x 2 y-blocks (MS=2048).

Strategy vs the bf16 baseline (169us):
  * Host passes x/y PRE-TRANSPOSED ([D, NS] / [D, MS]) as bf16 - pure
    layout/marshalling - so K lands on partitions with no device
    transposes (the baseline burned ~24us of DMA_TRANSPOSE + DRAM
    roundtrips).
  * Main dot runs in fp8 e4m3 with MatmulPerfMode.DoubleRow (2 k-tiles
    per instruction, 157 TF/s = 2x bf16).  xpT8 = fp8(xT * -2*rsx*inv_m1),
    ypT8 = fp8(yT * rsy), quantized on DVE against partition-broadcast
    scale rows.
  * |x|^2 row comes from an ones-matmul contraction (bf16) over ACT-squared
    xT tiles; the rank-1 extension terms stay high precision as a 7-row
    fp16 hi/lo matmul (K=8) appended to each psum accumulation group.
  * Vector row/layout shuffles go through tiny DRAM roundtrips; partition
    broadcasts use nc.gpsimd.partition_broadcast.
  * Mask sigmoid linearized as in the baseline: with 1/sqrt(m1) folded in,
    epilogue is r = Recip(psum) -> fp16, out = (r + c0)*r on DVE (16-bit
    2x mode), stored as fp16 (host upcasts to f32).
  * Numpy-simulated end-to-end max rel err of this numeric scheme: 1.0e-2
    (gate 2e-2); measured baseline was 6.2e-4.
"""

import os
import sys

sys.path.insert(0, "/opt/trn_rl_repo")

import numpy as np
import ml_dtypes

N, M, D, S = 8192, 4096, 512, 16
XB, YB = 4, 2  # core grid
CORES = XB * YB
NS = N // XB  # 2048 rows of x per core
MS = M // YB  # 2048 rows of y per core
PO = NS // 128  # 16 i-tiles
BW = MS // 128  # 16 = free-dim count per partition for y-side vectors
JT = MS // 512  # 4 j-tiles of 512
KT = D // 128  # 4 k-tiles
KQ = KT // 2  # 2 DoubleRow k-pair groups

SOFTPLUS_MIN = 1e-10
SOFTPLUS_MAX = 10000.0

_CACHE = {}

ACT_RECIP = bool(int(os.environ.get("ACT_RECIP", "1")))


def _act_recip(nc, out, in_):
    import concourse.mybir as mybir

    eng = nc.scalar
    inputs = [eng.lower_ap(in_)]
    for arg in (0.0, 1.0, 0.0):  # bias, scale, alpha
        inputs.append(mybir.ImmediateValue(dtype=mybir.dt.float32, value=arg))
    return eng.add_instruction(
        mybir.InstActivation(
            name=nc.get_next_instruction_name(),
            func=mybir.ActivationFunctionType.Reciprocal,
            ins=inputs,
            outs=[eng.lower_ap(out)],
        )
    )


def _fit_mask_linear(phi_val, cutoff_val, R=0.15):
    # res*sigmoid(phi*(res-c)) ~= m0*res + m1*res^2 for res in [0,R]:
    # linear chebyshev fit of g(t) = sigmoid(phi*(t-c)) on [0,R]
    t = (np.cos(np.linspace(0, np.pi, 2001)) + 1) * (R / 2)
    g = 1.0 / (1.0 + np.exp(-phi_val * (t - cutoff_val)))
    m1_, m0_ = np.polyfit(t, g, 1)
    gerr = np.abs(np.polyval([m1_, m0_], t) - g) / np.abs(g)
    assert gerr.max() < 2e-3, f"mask linearization too coarse: {gerr.max()}"
    return float(m0_), float(m1_)


def _build(phi_val: float, cutoff_val: float, R: float = 0.15):
    import concourse.mybir as mybir
    import concourse.tile as tile
    from concourse import bacc

    dt = mybir.dt
    AF = mybir.ActivationFunctionType
    OP = mybir.AluOpType
    DR = mybir.MatmulPerfMode.DoubleRow

    m0, m1 = _fit_mask_linear(phi_val, cutoff_val, R)
    # fold 1/sqrt(m1) into the matmul so the epilogue is
    #   r = recip(wtil/sqrt(m1)) = sqrt(m1)*res
    #   out = (r + m0/sqrt(m1)) * r = m1*res^2 + m0*res
    inv_m1 = 1.0 / float(np.sqrt(m1))
    c0 = m0 / float(np.sqrt(m1))

    nc = bacc.Bacc("TRN2", target_bir_lowering=False)

    xT_d = nc.dram_tensor("xT_shard", [D, NS], dt.bfloat16, kind="ExternalInput")
    yT_d = nc.dram_tensor("yT_shard", [D, MS], dt.bfloat16, kind="ExternalInput")
    sx_d = nc.dram_tensor("sample_x_shard", [NS, S], dt.float32, kind="ExternalInput")
    sy_d = nc.dram_tensor("sample_y_shard", [MS, S], dt.float32, kind="ExternalInput")
    sc_d = nc.dram_tensor("scale_full", [1, S], dt.float32, kind="ExternalInput")
    out_d = nc.dram_tensor("out_shard", [NS, MS], dt.float16, kind="ExternalOutput")

    # DRAM views. x rows use i = pi*PO + po (pi = partition) so that
    # [128, PO] -> flat-i DMA writes are contiguous 64B runs per partition;
    # y rows use j = a*BW + b likewise. xT/yT/out keep natural ordering.
    xT_v = xT_d.rearrange("(kt kp) i -> kp kt i", kp=128)  # [128, KT, NS]
    yT_v = yT_d.rearrange("(kt kp) j -> kp kt j", kp=128)  # [128, KT, MS]
    sx_v = sx_d.rearrange("(pi po) s -> pi po s", pi=128)  # [128, PO, S]
    sy_v = sy_d.rearrange("(a b) s -> a b s", a=128)  # [128, BW, S]
    out_v = out_d.rearrange("(po pi) j -> pi po j", pi=128)  # [128, PO, MS]

    with tile.TileContext(nc) as tc:
        with (
            tc.tile_pool(name="persist", bufs=1) as persist,
            tc.tile_pool(name="dram", bufs=1, space="DRAM") as dram,
            tc.tile_pool(name="psum", bufs=2, space="PSUM") as psum_p,
            tc.tile_pool(name="sqscr", bufs=2) as sqscr,
            tc.tile_pool(name="main", bufs=3) as main,
        ):
            # ---------------- scale broadcast ----------------
            sc_sb = persist.tile([1, S], dt.float32)
            nc.sync.dma_start(sc_sb[:], sc_d[:, :])
            scale_rep = persist.tile([128, S], dt.float32)
            nc.gpsimd.partition_broadcast(scale_rep[:], sc_sb[:])

            # ---------------- softplus chains (x and y sides) ----------------
            def softplus_rsqrt(samp_view, width, tag):
                """returns rs = clip(softplus(samp @ scale))**-0.5  [128,width]"""
                ss = persist.tile([128, width, S], dt.float32, tag=f"ss_{tag}")
                nc.sync.dma_start(ss[:], samp_view)
                tmp = persist.tile([128, width, S], dt.float32, tag=f"tmp_{tag}")
                nc.vector.tensor_tensor(
                    tmp[:], ss[:],
                    scale_rep[:, None, :].to_broadcast((128, width, S)), OP.mult,
                )
                red = persist.tile([128, width], dt.float32, tag=f"red_{tag}")
                nc.vector.tensor_reduce(
                    red[:, :, None], tmp[:], mybir.AxisListType.X, OP.add
                )
                v = persist.tile([128, width], dt.float32, tag=f"v_{tag}")
                nc.scalar.activation(v[:], red[:], AF.Exp)
                nc.scalar.activation(v[:], v[:], AF.Ln, bias=1.0)
                nc.vector.tensor_scalar(
                    v[:], v[:], SOFTPLUS_MAX, SOFTPLUS_MIN, OP.min, OP.max
                )
                rs = persist.tile([128, width], dt.float32, tag=f"rs_{tag}")
                nc.scalar.activation(rs[:], v[:], AF.Ln)
                nc.scalar.activation(rs[:], rs[:], AF.Exp, scale=-0.5)
                return rs

            rsx = softplus_rsqrt(sx_v, PO, "x")  # [128, PO], i = pi*PO+po
            rsy = softplus_rsqrt(sy_v, BW, "y")  # [128, BW], j = a*BW+b

            # ---------------- quant scale rows -> bcast tiles ----------------
            # x quant scale: -2*inv_m1*rsx ; y quant scale: rsy
            rxn2 = persist.tile([128, PO], dt.bfloat16)
            nc.vector.tensor_scalar_mul(rxn2[:], rsx[:], -2.0 * inv_m1)
            ry_bf = persist.tile([128, BW], dt.bfloat16)
            nc.vector.tensor_copy(ry_bf[:], rsy[:])

            d_rxn2 = dram.tile([1, NS], dt.bfloat16)
            d_ry = dram.tile([1, MS], dt.bfloat16)
            nc.sync.dma_start(
                d_rxn2[0, :].rearrange("(pi po) -> pi po", pi=128), rxn2[:]
            )
            nc.sync.dma_start(d_ry[0, :].rearrange("(a b) -> a b", a=128), ry_bf[:])

            rxn2_row = persist.tile([1, NS], dt.bfloat16)
            nc.sync.dma_start(rxn2_row[:], d_rxn2[:, :])
            ry_row = persist.tile([1, MS], dt.bfloat16)
            nc.sync.dma_start(ry_row[:], d_ry[:, :])

            rxn2_bc = persist.tile([128, NS], dt.bfloat16)
            nc.gpsimd.partition_broadcast(rxn2_bc[:], rxn2_row[:])
            ry_bc = persist.tile([128, MS], dt.bfloat16)
            nc.gpsimd.partition_broadcast(ry_bc[:], ry_row[:])

            # ---------------- load xT/yT, square + quantize ----------------
            xT_sb = persist.tile([128, KT, NS], dt.bfloat16)
            yT_sb = persist.tile([128, KT, MS], dt.bfloat16)
            for kt in range(KT):
                nc.sync.dma_start(xT_sb[:, kt, :], xT_v[:, kt, :])
                nc.sync.dma_start(yT_sb[:, kt, :], yT_v[:, kt, :])

            ones_bf = persist.tile([128, 128], dt.bfloat16)
            nc.vector.memset(ones_bf[:], 1.0)

            xp8 = persist.tile([128, KT, NS], dt.float8e4)
            yp8 = persist.tile([128, KT, MS], dt.float8e4)
            sqx_ps = psum_p.tile([128, 2048], dt.float32, tag="acc", name="sqx_ps")
            sqy_ps = psum_p.tile([128, 2048], dt.float32, tag="acc", name="sqy_ps")
            for kt in range(KT):
                # x: quantize (DVE) + square (ACT, bf16 out) + ones-contract
                nc.vector.tensor_tensor(
                    xp8[:, kt, :], xT_sb[:, kt, :], rxn2_bc[:], OP.mult
                )
                sq_scr = sqscr.tile([128, NS], dt.bfloat16, tag="sq")
                nc.scalar.activation(sq_scr[:], xT_sb[:, kt, :], AF.Square)
                for c in range(NS // 512):
                    nc.tensor.matmul(
                        sqx_ps[:, c * 512:(c + 1) * 512],
                        lhsT=ones_bf[:],
                        rhs=sq_scr[:, c * 512:(c + 1) * 512],
                        start=(kt == 0),
                        stop=(kt == KT - 1),
                    )
                # y
                nc.vector.tensor_tensor(
                    yp8[:, kt, :], yT_sb[:, kt, :], ry_bc[:], OP.mult
                )
                sq_scr_y = sqscr.tile([128, MS], dt.bfloat16, tag="sq")
                nc.scalar.activation(sq_scr_y[:], yT_sb[:, kt, :], AF.Square)
                for c in range(MS // 512):
                    nc.tensor.matmul(
                        sqy_ps[:, c * 512:(c + 1) * 512],
                        lhsT=ones_bf[:],
                        rhs=sq_scr_y[:, c * 512:(c + 1) * 512],
                        start=(kt == 0),
                        stop=(kt == KT - 1),
                    )

            # sq rows (every psum partition holds the same row) -> DRAM -> small
            sq_row_x = persist.tile([1, NS], dt.float32)
            nc.scalar.activation(sq_row_x[:], sqx_ps[0:1, :], AF.Copy)
            sq_row_y = persist.tile([1, MS], dt.float32)
            nc.vector.tensor_copy(sq_row_y[:], sqy_ps[0:1, :])
            d_sqx = dram.tile([1, NS], dt.float32)
            d_sqy = dram.tile([1, MS], dt.float32)
            nc.sync.dma_start(d_sqx[:, :], sq_row_x[:])
            nc.sync.dma_start(d_sqy[:, :], sq_row_y[:])
            sqx_pp = persist.tile([128, PO], dt.float32)
            nc.sync.dma_start(
                sqx_pp[:], d_sqx[0, :].rearrange("(pi po) -> pi po", pi=128)
            )
            sqy_pp = persist.tile([128, BW], dt.float32)
            nc.sync.dma_start(
                sqy_pp[:], d_sqy[0, :].rearrange("(a b) -> a b", a=128)
            )

            # ---------------- extension row vectors (fp16 hi/lo) ----------------
            def hi_lo16(vec, width, tag):
                hi = persist.tile([128, width], dt.float16, tag=f"{tag}_h")
                nc.vector.tensor_copy(hi[:], vec[:])
                hi_f = persist.tile([128, width], dt.float32, tag=f"{tag}_hf")
                nc.vector.tensor_copy(hi_f[:], hi[:])
                lo = persist.tile([128, width], dt.float16, tag=f"{tag}_l")
                nc.vector.tensor_tensor(lo[:], vec[:], hi_f[:], OP.subtract)
                return hi, lo

            # a = sqx * rsx * inv_m1 ; r = rsx * inv_m1   (x side, [128, PO])
            a_s = persist.tile([128, PO], dt.float32)
            nc.vector.tensor_tensor(a_s[:], sqx_pp[:], rsx[:], OP.mult)
            nc.vector.tensor_scalar_mul(a_s[:], a_s[:], inv_m1)
            r_s = persist.tile([128, PO], dt.float32)
            nc.vector.tensor_scalar_mul(r_s[:], rsx[:], inv_m1)
            a_hi, a_lo = hi_lo16(a_s, PO, "a")
            r_hi, r_lo = hi_lo16(r_s, PO, "r")

            # b = sqy * rsy ; rsy  (y side, [128, BW])
            b_s = persist.tile([128, BW], dt.float32)
            nc.vector.tensor_tensor(b_s[:], sqy_pp[:], rsy[:], OP.mult)
            ry_hi, ry_lo = hi_lo16(rsy, BW, "ry")
            b_hi, b_lo = hi_lo16(b_s, BW, "b")

            const_l = persist.tile([1, NS], dt.float16)
            nc.vector.memset(const_l[:], inv_m1)
            const_r = persist.tile([1, MS], dt.float16)
            nc.vector.memset(const_r[:], 1.0)

            d_vx = dram.tile([5, NS], dt.float16)
            for r, src in enumerate([a_hi, a_lo, r_hi, r_lo]):
                nc.sync.dma_start(
                    d_vx[r, :].rearrange("(pi po) -> pi po", pi=128), src[:]
                )
            nc.sync.dma_start(d_vx[4:5, :], const_l[:])
            d_vy = dram.tile([5, MS], dt.float16)
            for r, src in enumerate([ry_hi, ry_lo, b_hi, b_lo]):
                nc.sync.dma_start(
                    d_vy[r, :].rearrange("(a b) -> a b", a=128), src[:]
                )
            nc.sync.dma_start(d_vy[4:5, :], const_r[:])

            # ext lhsT rows [8, NS]:   0 a_hi, 1 a_hi, 2 a_lo, 3 r_hi, 4 r_hi,
            #                          5 r_lo, 6 inv_m1, 7 zero
            # ext rhs  rows [8, MS]:   0 ry_hi, 1 ry_lo, 2 ry_hi, 3 b_hi,
            #                          4 b_lo, 5 b_hi, 6 one, 7 zero
            ext_l = persist.tile([8, NS], dt.float16)
            nc.vector.memset(ext_l[:], 0.0)
            for r, v in enumerate([0, 0, 1, 2, 2, 3, 4]):
                nc.sync.dma_start(ext_l[r:r + 1, :], d_vx[v:v + 1, :])
            ext_r = persist.tile([8, MS], dt.float16)
            nc.vector.memset(ext_r[:], 0.0)
            for r, v in enumerate([0, 1, 0, 2, 3, 2, 4]):
                nc.sync.dma_start(ext_r[r:r + 1, :], d_vy[v:v + 1, :])

            # ---------------- main loop ----------------
            for po in range(PO):
                acc = psum_p.tile([128, 2048], dt.float32, tag="acc",
                                  name=f"acc{po}")
                for q in range(KQ):
                    for jt in range(JT):
                        nc.tensor.matmul(
                            acc[:, jt * 512:(jt + 1) * 512],
                            lhsT=xp8[:, 2 * q:2 * q + 2, po * 128:(po + 1) * 128],
                            rhs=yp8[:, 2 * q:2 * q + 2, jt * 512:(jt + 1) * 512],
                            start=(q == 0),
                            stop=False,
                            perf_mode=DR,
                        )
                for jt in range(JT):
                    nc.tensor.matmul(
                        acc[:, jt * 512:(jt + 1) * 512],
                        lhsT=ext_l[:, po * 128:(po + 1) * 128],
                        rhs=ext_r[:, jt * 512:(jt + 1) * 512],
                        start=False,
                        stop=True,
                    )
                r16 = main.tile([128, 2048], dt.float16, tag="r16")
                if ACT_RECIP:
                    _act_recip(nc, r16[:], acc[:])
                else:
                    rf = main.tile([128, 2048], dt.float32, tag="rf")
                    nc.vector.reciprocal_approx_fast(rf[:], acc[:])
                    nc.vector.tensor_copy(r16[:], rf[:])
                ot = main.tile([128, 2048], dt.float16, tag="ot")
                nc.vector.scalar_tensor_tensor(
                    ot[:], r16[:], c0, r16[:], OP.add, OP.mult
                )
                nc.sync.dma_start(out_v[:, po, :], ot[:])

    nc.compile()
    return nc


def _estimate_R(x, y, sample_x, sample_y, scale):
    # estimate the res range on a host-side subsample so the mask
    # linearization interval is snug (error grows with R^2)
    rng = np.random.default_rng(12345)
    ii = rng.integers(0, x.shape[0], 4096)
    jj = rng.integers(0, y.shape[0], 4096)
    xs = np.asarray(x)[ii].astype(np.float64)
    ys = np.asarray(y)[jj].astype(np.float64)
    dd = ((xs - ys) ** 2).sum(axis=1)
    sxs = np.clip(
        np.log1p(np.exp(np.asarray(sample_x)[ii].astype(np.float64)
                        @ np.asarray(scale).reshape(-1))),
        SOFTPLUS_MIN, SOFTPLUS_MAX,
    )
    sys_ = np.clip(
        np.log1p(np.exp(np.asarray(sample_y)[jj].astype(np.float64)
                        @ np.asarray(scale).reshape(-1))),
        SOFTPLUS_MIN, SOFTPLUS_MAX,
    )
    res_s = 1.0 / (1.0 + dd / np.sqrt(sxs * sys_))
    return float(min(1.0, max(3.0 * res_s.max(), 0.01)))


def kernel(x, y, sample_x, sample_y, scale, cutoff, phi):
    from concourse.bass_utils import run_bass_kernel_spmd

    phi_val = float(np.asarray(phi).reshape(-1)[0])
    cutoff_val = float(np.clip(np.asarray(cutoff).reshape(-1)[0], 0.0, 1000.0))
    R = _estimate_R(x, y, sample_x, sample_y, scale)

    key = (phi_val, cutoff_val, round(np.log2(R), 1))
    if key not in _CACHE:
        _CACHE[key] = _build(phi_val, cutoff_val, R)
    nc = _CACHE[key]

    x = np.asarray(x, dtype=np.float32)
    y = np.asarray(y, dtype=np.float32)
    sample_x = np.ascontiguousarray(np.asarray(sample_x, dtype=np.float32))
    sample_y = np.ascontiguousarray(np.asarray(sample_y, dtype=np.float32))
    scale = np.ascontiguousarray(np.asarray(scale, dtype=np.float32)).reshape(1, S)

    bf16 = ml_dtypes.bfloat16
    xT_shards = [
        x[cx * NS:(cx + 1) * NS].T.astype(bf16) for cx in range(XB)
    ]  # [D, NS] contiguous
    yT_shards = [y[cy * MS:(cy + 1) * MS].T.astype(bf16) for cy in range(YB)]
    sx_shards = [sample_x[cx * NS:(cx + 1) * NS] for cx in range(XB)]
    sy_shards = [sample_y[cy * MS:(cy + 1) * MS] for cy in range(YB)]

    in_maps = []
    for c in range(CORES):
        cx, cy = divmod(c, YB)
        in_maps.append(
            {
                "xT_shard": xT_shards[cx],
                "yT_shard": yT_shards[cy],
                "sample_x_shard": sx_shards[cx],
                "sample_y_shard": sy_shards[cy],
                "scale_full": scale,
            }
        )

    trace = bool(int(os.environ.get("KERNEL_TRACE", "0")))
    r = run_bass_kernel_spmd(nc, in_maps, core_ids=list(range(CORES)), trace=trace)
    kernel.last_results = r
    out = np.empty((N, M), dtype=np.float32)
    for c in range(CORES):
        cx, cy = divmod(c, YB)
        out[cx * NS:(cx + 1) * NS, cy * MS:(cy + 1) * MS] = (
            r.results[c]["out_shard"].astype(np.float32)
        )
    return out


if __name__ == "__main__":
    rng = np.random.default_rng(0)
    ins = {
        "x": rng.standard_normal((N, D), dtype=np.float32),
        "y": rng.standard_normal((M, D), dtype=np.float32),
        "sample_x": rng.random((N, S), dtype=np.float32),
        "sample_y": rng.random((M, S), dtype=np.float32),
        "scale": rng.random((S,), dtype=np.float32),
        "cutoff": np.full((1,), 0.1, dtype=np.float32),
        "phi": np.ones((1,), dtype=np.float32),
    }
    o = kernel(**ins)
    print(o.shape, o.dtype, o[:2, :4])
